# Initial kernel scaffold
#
"""DeepSeek-style MoE (8 experts, top-2, shared expert) on 8 Trainium2 cores.

Strategy (expert-parallel, host dispatch/combine):
  - Host computes routing (gate scores, top-k, combine weights) in numpy;
    the top-2 margins for this problem are ~1e-4+ so any fp32 evaluation
    selects identical experts to the jax reference.
  - Core e receives the tokens routed to expert e (gathered, transposed to
    [H, C] feature-major layout) plus expert e's weights, and a 1/8 token
    slice for the (replicated-weight) shared expert.
  - Each core runs both swiglu stacks with fp32r matmuls in transposed
    layout (weights stationary, tokens moving), so gate/up outputs land
    directly in the layout the down-projection consumes — no transposes.
  - Host scales expert outputs by combine weights and scatter-adds.
"""

import numpy as np

import concourse.bacc as bacc
import concourse.mybir as mybir
from concourse.bass_utils import run_bass_kernel_spmd
from concourse.tile import TileContext

B, S, H, I, E, TOPK = 4, 1024, 1024, 2048, 8, 2
T = B * S
NCORES = 8
TOKS_SH = T // NCORES  # shared-expert tokens per core
F32 = mybir.dt.float32
F32R = mybir.dt.float32r

HT = H // 128   # 8 h-tiles
IT = I // 128   # 16 i-tiles
TOK_TILE = 512  # moving-dim tile (max for 4-byte dtypes)

_kernel_cache: dict[int, object] = {}


def _tok_tiles(n):
    """Split n tokens into moving-dim tiles of 512 (tail >= 256)."""
    out = []
    t = 0
    while t < n:
        w = min(TOK_TILE, n - t)
        out.append((t, w))
        t += w
    return out


def _emit_swiglu(nc, tc, pools, xsb, vts, wg_d, wu_d, wd_d, y_d, n_tok):
    """Emit one transposed swiglu stack: y^T = (silu(x@Wg) * (x@Wu)) @ Wd, all
    operands/results in feature-major [feature, token] layout.

    xsb: 8 SBUF tiles [128, >=n_tok] (f32r) holding x^T h-tiles.
    vts: 16 SBUF tiles [128, >=n_tok] (f32r) used for the intermediate v^T.
    """
    wpool, spool, pp = pools
    tiles = _tok_tiles(n_tok)

    # Phase 1: v^T[i] = silu(Wg[:,i].T @ x^T) * (Wu[:,i].T @ x^T)
    for i in range(IT):
        wg_t = wpool.tile([128, HT, 128], F32R, tag="wg", name=f"wg{i}")
        wu_t = wpool.tile([128, HT, 128], F32R, tag="wu", name=f"wu{i}")
        src_g = wg_d[:, i * 128:(i + 1) * 128].bitcast(F32R).rearrange(
            "(h p) c -> p h c", p=128)
        src_u = wu_d[:, i * 128:(i + 1) * 128].bitcast(F32R).rearrange(
            "(h p) c -> p h c", p=128)
        nc.sync.dma_start(wg_t[:], src_g)
        nc.sync.dma_start(wu_t[:], src_u)
        for t0, tw in tiles:
            g_ps = pp.tile([128, TOK_TILE], F32, tag="gps", name=f"g{i}_{t0}")
            u_ps = pp.tile([128, TOK_TILE], F32, tag="ups", name=f"u{i}_{t0}")
            for h in range(HT):
                nc.tensor.matmul(
                    g_ps[:, :tw], wg_t[:, h, :], xsb[h][:, t0:t0 + tw],
                    start=(h == 0), stop=(h == HT - 1))
            for h in range(HT):
                nc.tensor.matmul(
                    u_ps[:, :tw], wu_t[:, h, :], xsb[h][:, t0:t0 + tw],
                    start=(h == 0), stop=(h == HT - 1))
            s_t = spool.tile([128, TOK_TILE], F32, tag="silu", name=f"s{i}_{t0}")
            nc.scalar.activation(
                s_t[:, :tw], g_ps[:, :tw], mybir.ActivationFunctionType.Silu)
            nc.vector.tensor_mul(vts[i][:, t0:t0 + tw], s_t[:, :tw], u_ps[:, :tw])

    # Phase 2: y^T[h] = sum_i Wd[i,h].T @ v^T[i]
    for h in range(HT):
        wd_t = wpool.tile([128, IT, 128], F32R, tag="wd", name=f"wd{h}")
        src_d = wd_d[:, h * 128:(h + 1) * 128].bitcast(F32R).rearrange(
            "(i p) c -> p i c", p=128)
        nc.sync.dma_start(wd_t[:], src_d)
        for t0, tw in tiles:
            y_ps = pp.tile([128, TOK_TILE], F32, tag="yps", name=f"y{h}_{t0}")
            for i in range(IT):
                nc.tensor.matmul(
                    y_ps[:, :tw], wd_t[:, i, :], vts[i][:, t0:t0 + tw],
                    start=(i == 0), stop=(i == IT - 1))
            y_sb = spool.tile([128, TOK_TILE], F32, tag="yout", name=f"yo{h}_{t0}")
            nc.vector.tensor_copy(y_sb[:, :tw], y_ps[:, :tw])
            nc.sync.dma_start(y_d[h * 128:(h + 1) * 128, t0:t0 + tw], y_sb[:, :tw])


def _build(C):
    nc = bacc.Bacc(None, target_bir_lowering=False, debug=False)
    xg_d = nc.declare_dram_parameter("xg", [H, C], F32, isOutput=False)
    xs_d = nc.declare_dram_parameter("xs", [H, TOKS_SH], F32, isOutput=False)
    wg_d = nc.declare_dram_parameter("wg", [H, I], F32, isOutput=False)
    wu_d = nc.declare_dram_parameter("wu", [H, I], F32, isOutput=False)
    wd_d = nc.declare_dram_parameter("wd", [I, H], F32, isOutput=False)
    sg_d = nc.declare_dram_parameter("sg", [H, I], F32, isOutput=False)
    su_d = nc.declare_dram_parameter("su", [H, I], F32, isOutput=False)
    sd_d = nc.declare_dram_parameter("sd", [I, H], F32, isOutput=False)
    yg_d = nc.declare_dram_parameter("yg", [H, C], F32, isOutput=True)
    ys_d = nc.declare_dram_parameter("ys", [H, TOKS_SH], F32, isOutput=True)

    with TileContext(nc) as tc:
        with (
            tc.tile_pool(name="xpool", bufs=1) as xpool,
            tc.tile_pool(name="vpool", bufs=1) as vpool,
            tc.tile_pool(name="wpool", bufs=2) as wpool,
            tc.tile_pool(name="spool", bufs=3) as spool,
            tc.tile_pool(name="psum", bufs=2, space="PSUM") as pp,
        ):
            xsb = [xpool.tile([128, C], F32R, tag=f"x{h}", name=f"x{h}")
                   for h in range(HT)]
            vts = [vpool.tile([128, C], F32R, tag=f"v{i}", name=f"v{i}")
                   for i in range(IT)]
            for h in range(HT):
                nc.sync.dma_start(
                    xsb[h][:], xg_d[h * 128:(h + 1) * 128, :].bitcast(F32R))

            pools = (wpool, spool, pp)
            _emit_swiglu(nc, tc, pools, xsb, vts, wg_d, wu_d, wd_d, yg_d, C)

            # Shared expert on this core's 1/8 token slice; reuse x/v tiles.
            for h in range(HT):
                nc.sync.dma_start(
                    xsb[h][:, :TOKS_SH],
                    xs_d[h * 128:(h + 1) * 128, :].bitcast(F32R))
            _emit_swiglu(nc, tc, pools, xsb, vts, sg_d, su_d, sd_d, ys_d,
                         TOKS_SH)
    nc.compile()
    return nc


def _routing(xf, gate_w, expert_bias):
    scores = xf @ gate_w.T  # [T, E] fp32
    biased = scores + expert_bias[None, :]
    # top-k by value, ties broken by lower index (matches jax.lax.top_k)
    order = np.argsort(-biased, axis=1, kind="stable")
    idx = order[:, :TOPK]
    rp = -np.sort(-scores, axis=1, kind="stable")[:, :TOPK]
    w = 1.0 / (1.0 + np.exp(-rp, dtype=np.float32))
    w = w / w.sum(axis=1, keepdims=True)
    cw = np.zeros((T, E), np.float32)
    for j in range(TOPK):
        cw[np.arange(T), idx[:, j]] += w[:, j]
    counts = np.bincount(idx.reshape(-1), minlength=E)
    freq = (counts / idx.size).astype(np.float32)
    viol = np.float32((freq.max() - np.float32(freq.mean())) / np.float32(freq.mean()))
    return idx, cw, counts, viol


def kernel(x, gate_w, expert_bias, Wg, Wu, Wd, Sg, Su, Sd):
    x = np.ascontiguousarray(np.asarray(x, np.float32))
    gate_w = np.asarray(gate_w, np.float32)
    expert_bias = np.asarray(expert_bias, np.float32)
    Wg = np.asarray(Wg, np.float32)
    Wu = np.asarray(Wu, np.float32)
    Wd = np.asarray(Wd, np.float32)
    Sg = np.ascontiguousarray(np.asarray(Sg, np.float32))
    Su = np.ascontiguousarray(np.asarray(Su, np.float32))
    Sd = np.ascontiguousarray(np.asarray(Sd, np.float32))

    xf = x.reshape(T, H)
    idx, cw, counts, viol = _routing(xf, gate_w, expert_bias)

    # capacity: max expert load, padded to a multiple of 256 (>= 512)
    C = max(512, int(-(-counts.max() // 256)) * 256)

    if C not in _kernel_cache:
        _kernel_cache[C] = _build(C)
    nc = _kernel_cache[C]

    xfT = np.ascontiguousarray(xf.T)  # [H, T]
    tok_lists = []
    in_maps = []
    for e in range(E):
        toks = np.flatnonzero(cw[:, e] > 0.0)
        tok_lists.append(toks)
        xg = np.zeros((H, C), np.float32)
        xg[:, :len(toks)] = xfT[:, toks]
        in_maps.append({
            "xg": xg,
            "xs": np.ascontiguousarray(xfT[:, e * TOKS_SH:(e + 1) * TOKS_SH]),
            "wg": np.ascontiguousarray(Wg[e]),
            "wu": np.ascontiguousarray(Wu[e]),
            "wd": np.ascontiguousarray(Wd[e]),
            "sg": Sg, "su": Su, "sd": Sd,
        })

    res = run_bass_kernel_spmd(nc, in_maps, list(range(NCORES)))

    out = np.zeros((T, H), np.float32)
    for e in range(E):
        toks = tok_lists[e]
        yg = res.results[e]["yg"][:, :len(toks)]  # [H, n_e]
        out[toks] += cw[toks, e][:, None] * yg.T
    for e in range(E):
        out[e * TOKS_SH:(e + 1) * TOKS_SH] += res.results[e]["ys"].T
    return out.reshape(B, S, H), viol


# revision 5
# speedup vs baseline: 1.0993x; 1.0993x over previous
"""DeepSeek-style MoE (8 experts, top-2, shared expert) on 8 Trainium2 cores.

Strategy (expert-parallel, host dispatch/combine):
  - Host computes routing (gate scores, top-k, combine weights) in numpy;
    the top-2 margins for this problem are ~1e-4+ so any fp32 evaluation
    selects identical experts to the jax reference.
  - Core e receives the tokens routed to expert e (gathered, transposed to
    [H, C] feature-major layout) plus expert e's weights, and a 1/8 token
    slice for the (replicated-weight) shared expert.
  - Each core runs both swiglu stacks with fp32r matmuls in transposed
    layout (weights stationary, tokens moving), so gate/up outputs land
    directly in the layout the down-projection consumes — no transposes.
  - Host scales expert outputs by combine weights and scatter-adds.
"""

import numpy as np

import concourse.bacc as bacc
import concourse.mybir as mybir
from concourse.bass_utils import run_bass_kernel_spmd
from concourse.tile import TileContext

B, S, H, I, E, TOPK = 4, 1024, 1024, 2048, 8, 2
T = B * S
NCORES = 8
TOKS_SH = T // NCORES  # shared-expert tokens per core
F32 = mybir.dt.float32
F32R = mybir.dt.float32r

HT = H // 128   # 8 h-tiles
IT = I // 128   # 16 i-tiles
TOK_TILE = 256  # moving-dim tile: fp32r streams at ~99% peak at N=256
                # (N=512 internally splits into 2x256 with weight reloads)

_kernel_cache: dict[int, object] = {}


def _tok_tiles(n):
    """Split n tokens into moving-dim tiles of TOK_TILE."""
    out = []
    t = 0
    while t < n:
        w = min(TOK_TILE, n - t)
        out.append((t, w))
        t += w
    return out


def _emit_swiglu(nc, tc, pools, xsb, vts, wg_d, wu_d, wd_d, y_d, n_tok):
    """Emit one transposed swiglu stack: y^T = (silu(x@Wg) * (x@Wu)) @ Wd, all
    operands/results in feature-major [feature, token] layout.

    xsb: 8 SBUF tiles [128, >=n_tok] (f32r) holding x^T h-tiles.
    vts: 16 SBUF tiles [128, >=n_tok] (f32r) used for the intermediate v^T.
    """
    wpool, spool, pp = pools
    tiles = _tok_tiles(n_tok)

    # Phase 1: v^T[i] = silu(Wg[:,i].T @ x^T) * (Wu[:,i].T @ x^T)
    for i in range(IT):
        wg_t = wpool.tile([128, HT, 128], F32R, tag="wg", name=f"wg{i}")
        wu_t = wpool.tile([128, HT, 128], F32R, tag="wu", name=f"wu{i}")
        src_g = wg_d[:, i * 128:(i + 1) * 128].bitcast(F32R).rearrange(
            "(h p) c -> p h c", p=128)
        src_u = wu_d[:, i * 128:(i + 1) * 128].bitcast(F32R).rearrange(
            "(h p) c -> p h c", p=128)
        nc.sync.dma_start(wg_t[:], src_g)
        nc.sync.dma_start(wu_t[:], src_u)
        for t0, tw in tiles:
            g_ps = pp.tile([128, TOK_TILE], F32, tag="gps", name=f"g{i}_{t0}")
            u_ps = pp.tile([128, TOK_TILE], F32, tag="ups", name=f"u{i}_{t0}")
            for h in range(HT):
                nc.tensor.matmul(
                    g_ps[:, :tw], wg_t[:, h, :], xsb[h][:, t0:t0 + tw],
                    start=(h == 0), stop=(h == HT - 1))
            for h in range(HT):
                nc.tensor.matmul(
                    u_ps[:, :tw], wu_t[:, h, :], xsb[h][:, t0:t0 + tw],
                    start=(h == 0), stop=(h == HT - 1))
            s_t = spool.tile([128, TOK_TILE], F32, tag="silu", name=f"s{i}_{t0}")
            nc.scalar.activation(
                s_t[:, :tw], g_ps[:, :tw], mybir.ActivationFunctionType.Silu)
            nc.vector.tensor_mul(vts[i][:, t0:t0 + tw], s_t[:, :tw], u_ps[:, :tw])

    # Phase 2: y^T[h] = sum_i Wd[i,h].T @ v^T[i]
    for h in range(HT):
        wd_t = wpool.tile([128, IT, 128], F32R, tag="wd", name=f"wd{h}")
        src_d = wd_d[:, h * 128:(h + 1) * 128].bitcast(F32R).rearrange(
            "(i p) c -> p i c", p=128)
        nc.sync.dma_start(wd_t[:], src_d)
        for t0, tw in tiles:
            y_ps = pp.tile([128, TOK_TILE], F32, tag="yps", name=f"y{h}_{t0}")
            for i in range(IT):
                nc.tensor.matmul(
                    y_ps[:, :tw], wd_t[:, i, :], vts[i][:, t0:t0 + tw],
                    start=(i == 0), stop=(i == IT - 1))
            y_sb = spool.tile([128, TOK_TILE], F32, tag="yout", name=f"yo{h}_{t0}")
            nc.vector.tensor_copy(y_sb[:, :tw], y_ps[:, :tw])
            nc.sync.dma_start(y_d[h * 128:(h + 1) * 128, t0:t0 + tw], y_sb[:, :tw])


def _build(C, loops=1):
    nc = bacc.Bacc(None, target_bir_lowering=False, debug=False)
    xg_d = nc.declare_dram_parameter("xg", [H, C], F32, isOutput=False)
    xs_d = nc.declare_dram_parameter("xs", [H, TOKS_SH], F32, isOutput=False)
    wg_d = nc.declare_dram_parameter("wg", [H, I], F32, isOutput=False)
    wu_d = nc.declare_dram_parameter("wu", [H, I], F32, isOutput=False)
    wd_d = nc.declare_dram_parameter("wd", [I, H], F32, isOutput=False)
    sg_d = nc.declare_dram_parameter("sg", [H, I], F32, isOutput=False)
    su_d = nc.declare_dram_parameter("su", [H, I], F32, isOutput=False)
    sd_d = nc.declare_dram_parameter("sd", [I, H], F32, isOutput=False)
    yg_d = nc.declare_dram_parameter("yg", [H, C], F32, isOutput=True)
    ys_d = nc.declare_dram_parameter("ys", [H, TOKS_SH], F32, isOutput=True)

    from contextlib import ExitStack, nullcontext

    with TileContext(nc) as tc:
        with (
            tc.tile_pool(name="xpool", bufs=1) as xpool,
            tc.tile_pool(name="vpool", bufs=1) as vpool,
            tc.tile_pool(name="wpool", bufs=2) as wpool,
            tc.tile_pool(name="spool", bufs=3) as spool,
            tc.tile_pool(name="psum", bufs=2, space="PSUM") as pp,
        ):
            with (tc.For_i(0, loops, 1) if loops > 1 else nullcontext()):
                xsb = [xpool.tile([128, C], F32R, tag=f"x{h}", name=f"x{h}")
                       for h in range(HT)]
                vts = [vpool.tile([128, C], F32R, tag=f"v{i}", name=f"v{i}")
                       for i in range(IT)]
                for h in range(HT):
                    nc.sync.dma_start(
                        xsb[h][:], xg_d[h * 128:(h + 1) * 128, :].bitcast(F32R))

                pools = (wpool, spool, pp)
                _emit_swiglu(nc, tc, pools, xsb, vts, wg_d, wu_d, wd_d, yg_d, C)

                # Shared expert on this core's 1/8 token slice; reuse x/v tiles.
                for h in range(HT):
                    nc.sync.dma_start(
                        xsb[h][:, :TOKS_SH],
                        xs_d[h * 128:(h + 1) * 128, :].bitcast(F32R))
                _emit_swiglu(nc, tc, pools, xsb, vts, sg_d, su_d, sd_d, ys_d,
                             TOKS_SH)
    nc.compile()
    return nc


def _routing(xf, gate_w, expert_bias):
    scores = xf @ gate_w.T  # [T, E] fp32
    biased = scores + expert_bias[None, :]
    # top-k by value, ties broken by lower index (matches jax.lax.top_k)
    order = np.argsort(-biased, axis=1, kind="stable")
    idx = order[:, :TOPK]
    rp = -np.sort(-scores, axis=1, kind="stable")[:, :TOPK]
    w = 1.0 / (1.0 + np.exp(-rp, dtype=np.float32))
    w = w / w.sum(axis=1, keepdims=True)
    cw = np.zeros((T, E), np.float32)
    for j in range(TOPK):
        cw[np.arange(T), idx[:, j]] += w[:, j]
    counts = np.bincount(idx.reshape(-1), minlength=E)
    freq = (counts / idx.size).astype(np.float32)
    viol = np.float32((freq.max() - np.float32(freq.mean())) / np.float32(freq.mean()))
    return idx, cw, counts, viol


def kernel(x, gate_w, expert_bias, Wg, Wu, Wd, Sg, Su, Sd):
    x = np.ascontiguousarray(np.asarray(x, np.float32))
    gate_w = np.asarray(gate_w, np.float32)
    expert_bias = np.asarray(expert_bias, np.float32)
    Wg = np.asarray(Wg, np.float32)
    Wu = np.asarray(Wu, np.float32)
    Wd = np.asarray(Wd, np.float32)
    Sg = np.ascontiguousarray(np.asarray(Sg, np.float32))
    Su = np.ascontiguousarray(np.asarray(Su, np.float32))
    Sd = np.ascontiguousarray(np.asarray(Sd, np.float32))

    xf = x.reshape(T, H)
    idx, cw, counts, viol = _routing(xf, gate_w, expert_bias)

    # capacity: max expert load, padded to a multiple of 256 (>= 512)
    C = max(512, int(-(-counts.max() // 256)) * 256)

    if C not in _kernel_cache:
        _kernel_cache[C] = _build(C)
    nc = _kernel_cache[C]

    xfT = np.ascontiguousarray(xf.T)  # [H, T]
    tok_lists = []
    in_maps = []
    for e in range(E):
        toks = np.flatnonzero(cw[:, e] > 0.0)
        tok_lists.append(toks)
        xg = np.zeros((H, C), np.float32)
        xg[:, :len(toks)] = xfT[:, toks]
        in_maps.append({
            "xg": xg,
            "xs": np.ascontiguousarray(xfT[:, e * TOKS_SH:(e + 1) * TOKS_SH]),
            "wg": np.ascontiguousarray(Wg[e]),
            "wu": np.ascontiguousarray(Wu[e]),
            "wd": np.ascontiguousarray(Wd[e]),
            "sg": Sg, "su": Su, "sd": Sd,
        })

    res = run_bass_kernel_spmd(nc, in_maps, list(range(NCORES)))

    out = np.zeros((T, H), np.float32)
    for e in range(E):
        toks = tok_lists[e]
        yg = res.results[e]["yg"][:, :len(toks)]  # [H, n_e]
        out[toks] += cw[toks, e][:, None] * yg.T
    for e in range(E):
        out[e * TOKS_SH:(e + 1) * TOKS_SH] += res.results[e]["ys"].T
    return out.reshape(B, S, H), viol


# revision 8
# speedup vs baseline: 1.1373x; 1.0345x over previous
"""DeepSeek-style MoE (8 experts, top-2, shared expert) on 8 Trainium2 cores.

Strategy (expert-parallel, host dispatch/combine):
  - Host computes routing (gate scores, top-k, combine weights) in numpy;
    the top-2 margins for this problem are ~1e-4+ so any fp32 evaluation
    selects identical experts to the jax reference.
  - Core e receives the tokens routed to expert e (gathered, transposed to
    [H, C] feature-major layout) plus expert e's weights, and a 1/8 token
    slice for the (replicated-weight) shared expert.
  - Each core runs both swiglu stacks with fp32r matmuls in transposed
    layout (weights stationary, tokens moving), so gate/up outputs land
    directly in the layout the down-projection consumes — no transposes.
    fp32r at moving-dim 256 sustains ~99% of PE peak.
  - Weights are host-packed into [block, 128, 1024] tile-major layout so
    every weight DMA is 4KB-contiguous per partition.
  - Host scales expert outputs by combine weights and scatter-adds.
"""

from contextlib import nullcontext

import numpy as np

import concourse.bacc as bacc
import concourse.mybir as mybir
from concourse.bass_utils import run_bass_kernel_spmd
from concourse.tile import TileContext

B, S, H, I, E, TOPK = 4, 1024, 1024, 2048, 8, 2
T = B * S
NCORES = 8
TOKS_SH = T // NCORES  # shared-expert tokens per core
F32 = mybir.dt.float32
F32R = mybir.dt.float32r

HT = H // 128   # 8 h-tiles
IT = I // 128   # 16 i-tiles
TOK_TILE = 256  # moving-dim tile: fp32r streams at ~99% peak at N=256
                # (N=512 internally splits into 2x256 with weight reloads)

_kernel_cache: dict[int, object] = {}


def _tok_tiles(n):
    out = []
    t = 0
    while t < n:
        w = min(TOK_TILE, n - t)
        out.append((t, w))
        t += w
    return out


def _emit_swiglu(nc, tc, pools, xsb, vts, wg_d, wu_d, wd_d, y_d, n_tok,
                 x_loader=None):
    """One transposed swiglu stack: y^T = (silu(x@Wg) * (x@Wu)) @ Wd.

    xsb: 8 SBUF tiles [128, >=n_tok] (f32r) holding x^T h-tiles.
    vts: 16 SBUF tiles [128, >=n_tok] (f32r) for the intermediate v^T.
    wg_d/wu_d: packed [IT, 128, HT*128]; wd_d: packed [HT, 128, IT*128].
    x_loader(t_idx): emits the x DMAs for tok tile t (called during i==0).
    """
    wpool, spool, pp = pools
    tiles = _tok_tiles(n_tok)

    # Phase 1: v^T[i] = silu(Wg[:,i].T @ x^T) * (Wu[:,i].T @ x^T)
    for i in range(IT):
        wg_t = wpool.tile([128, HT * 128], F32R, tag="wg", name=f"wg{i}")
        wu_t = wpool.tile([128, HT * 128], F32R, tag="wu", name=f"wu{i}")
        nc.sync.dma_start(wg_t[:], wg_d[i].bitcast(F32R))
        nc.sync.dma_start(wu_t[:], wu_d[i].bitcast(F32R))
        for ti, (t0, tw) in enumerate(tiles):
            if i == 0 and x_loader is not None:
                x_loader(ti)
            g_ps = pp.tile([128, TOK_TILE], F32, tag="gps", name=f"g{i}_{t0}")
            u_ps = pp.tile([128, TOK_TILE], F32, tag="ups", name=f"u{i}_{t0}")
            for h in range(HT):
                nc.tensor.matmul(
                    g_ps[:, :tw], wg_t[:, h * 128:(h + 1) * 128],
                    xsb[h][:, t0:t0 + tw],
                    start=(h == 0), stop=(h == HT - 1))
            for h in range(HT):
                nc.tensor.matmul(
                    u_ps[:, :tw], wu_t[:, h * 128:(h + 1) * 128],
                    xsb[h][:, t0:t0 + tw],
                    start=(h == 0), stop=(h == HT - 1))
            s_t = spool.tile([128, TOK_TILE], F32, tag="silu", name=f"s{i}_{t0}")
            nc.scalar.activation(
                s_t[:, :tw], g_ps[:, :tw], mybir.ActivationFunctionType.Silu)
            nc.vector.tensor_mul(vts[i][:, t0:t0 + tw], s_t[:, :tw], u_ps[:, :tw])

    # Phase 2: y^T[h] = sum_i Wd[i,h].T @ v^T[i]
    for h in range(HT):
        wd_t = wpool.tile([128, IT * 128], F32R, tag="wd", name=f"wd{h}")
        nc.sync.dma_start(wd_t[:], wd_d[h].bitcast(F32R))
        for t0, tw in tiles:
            y_ps = pp.tile([128, TOK_TILE], F32, tag="yps", name=f"y{h}_{t0}")
            for i in range(IT):
                nc.tensor.matmul(
                    y_ps[:, :tw], wd_t[:, i * 128:(i + 1) * 128],
                    vts[i][:, t0:t0 + tw],
                    start=(i == 0), stop=(i == IT - 1))
            y_sb = spool.tile([128, TOK_TILE], F32, tag="yout", name=f"yo{h}_{t0}")
            nc.vector.tensor_copy(y_sb[:, :tw], y_ps[:, :tw])
            nc.sync.dma_start(y_d[h * 128:(h + 1) * 128, t0:t0 + tw], y_sb[:, :tw])


def _build(C, loops=1):
    nc = bacc.Bacc(None, target_bir_lowering=False, debug=False)
    xg_d = nc.declare_dram_parameter("xg", [H, C], F32, isOutput=False)
    xs_d = nc.declare_dram_parameter("xs", [H, TOKS_SH], F32, isOutput=False)
    wg_d = nc.declare_dram_parameter("wg", [IT, 128, HT * 128], F32, isOutput=False)
    wu_d = nc.declare_dram_parameter("wu", [IT, 128, HT * 128], F32, isOutput=False)
    wd_d = nc.declare_dram_parameter("wd", [HT, 128, IT * 128], F32, isOutput=False)
    sg_d = nc.declare_dram_parameter("sg", [IT, 128, HT * 128], F32, isOutput=False)
    su_d = nc.declare_dram_parameter("su", [IT, 128, HT * 128], F32, isOutput=False)
    sd_d = nc.declare_dram_parameter("sd", [HT, 128, IT * 128], F32, isOutput=False)
    yg_d = nc.declare_dram_parameter("yg", [H, C], F32, isOutput=True)
    ys_d = nc.declare_dram_parameter("ys", [H, TOKS_SH], F32, isOutput=True)

    with TileContext(nc) as tc:
        with (
            tc.tile_pool(name="xpool", bufs=1) as xpool,
            tc.tile_pool(name="vpool", bufs=1) as vpool,
            tc.tile_pool(name="wpool", bufs=3) as wpool,
            tc.tile_pool(name="spool", bufs=2) as spool,
            tc.tile_pool(name="psum", bufs=2, space="PSUM") as pp,
        ):
            with (tc.For_i(0, loops, 1) if loops > 1 else nullcontext()):
                xsb = [xpool.tile([128, C], F32R, tag=f"x{h}", name=f"x{h}")
                       for h in range(HT)]
                xsh = [xpool.tile([128, TOKS_SH], F32R, tag=f"xs{h}",
                                  name=f"xs{h}") for h in range(HT)]
                vts = [vpool.tile([128, C], F32R, tag=f"v{i}", name=f"v{i}")
                       for i in range(IT)]

                exp_tiles = _tok_tiles(C)

                def load_xg(ti):
                    t0, tw = exp_tiles[ti]
                    for h in range(HT):
                        nc.sync.dma_start(
                            xsb[h][:, t0:t0 + tw],
                            xg_d[h * 128:(h + 1) * 128, t0:t0 + tw].bitcast(F32R))
                    if ti == len(exp_tiles) - 1:
                        for h in range(HT):
                            nc.sync.dma_start(
                                xsh[h][:],
                                xs_d[h * 128:(h + 1) * 128, :].bitcast(F32R))

                pools = (wpool, spool, pp)
                _emit_swiglu(nc, tc, pools, xsb, vts, wg_d, wu_d, wd_d, yg_d,
                             C, x_loader=load_xg)
                _emit_swiglu(nc, tc, pools, xsh, vts, sg_d, su_d, sd_d, ys_d,
                             TOKS_SH)
    nc.compile()
    return nc


def _pack_gu(w):
    """[H, I] -> [IT, 128, HT*128]: block i, partition p, col h*128+c =
    w[h*128+p, i*128+c] (tile-contiguous weight layout)."""
    return np.ascontiguousarray(
        w.reshape(HT, 128, IT, 128).transpose(2, 1, 0, 3).reshape(IT, 128, HT * 128))


def _pack_d(w):
    """[I, H] -> [HT, 128, IT*128]."""
    return np.ascontiguousarray(
        w.reshape(IT, 128, HT, 128).transpose(2, 1, 0, 3).reshape(HT, 128, IT * 128))


def _routing(xf, gate_w, expert_bias):
    scores = xf @ np.asarray(gate_w, np.float32).T  # [T, E]
    biased = scores + np.asarray(expert_bias, np.float32)[None, :]
    order = np.argsort(-biased, axis=1, kind="stable")
    idx = order[:, :TOPK]
    rp = -np.sort(-scores, axis=1, kind="stable")[:, :TOPK]
    w = 1.0 / (1.0 + np.exp(-rp, dtype=np.float32))
    w = w / w.sum(axis=1, keepdims=True)
    cw = np.zeros((T, E), np.float32)
    for j in range(TOPK):
        cw[np.arange(T), idx[:, j]] += w[:, j]
    counts = np.bincount(idx.reshape(-1), minlength=E)
    freq = (counts / idx.size).astype(np.float32)
    viol = np.float32((freq.max() - np.float32(freq.mean())) / np.float32(freq.mean()))
    return idx, cw, counts, viol


def kernel(x, gate_w, expert_bias, Wg, Wu, Wd, Sg, Su, Sd):
    x = np.ascontiguousarray(np.asarray(x, np.float32))
    Wg = np.asarray(Wg, np.float32)
    Wu = np.asarray(Wu, np.float32)
    Wd = np.asarray(Wd, np.float32)

    xf = x.reshape(T, H)
    idx, cw, counts, viol = _routing(xf, gate_w, expert_bias)

    # capacity: max expert load, padded to a multiple of 256 (>= 512)
    C = max(512, int(-(-counts.max() // 256)) * 256)

    if C not in _kernel_cache:
        _kernel_cache[C] = _build(C)
    nc = _kernel_cache[C]

    sgp = _pack_gu(np.asarray(Sg, np.float32))
    sup = _pack_gu(np.asarray(Su, np.float32))
    sdp = _pack_d(np.asarray(Sd, np.float32))

    xfT = np.ascontiguousarray(xf.T)  # [H, T]
    tok_lists = []
    in_maps = []
    for e in range(E):
        toks = np.flatnonzero(cw[:, e] > 0.0)
        tok_lists.append(toks)
        xg = np.zeros((H, C), np.float32)
        xg[:, :len(toks)] = xfT[:, toks]
        in_maps.append({
            "xg": xg,
            "xs": np.ascontiguousarray(xfT[:, e * TOKS_SH:(e + 1) * TOKS_SH]),
            "wg": _pack_gu(Wg[e]),
            "wu": _pack_gu(Wu[e]),
            "wd": _pack_d(Wd[e]),
            "sg": sgp, "su": sup, "sd": sdp,
        })

    res = run_bass_kernel_spmd(nc, in_maps, list(range(NCORES)))

    out = np.zeros((T, H), np.float32)
    for e in range(E):
        toks = tok_lists[e]
        yg = res.results[e]["yg"][:, :len(toks)]  # [H, n_e]
        out[toks] += cw[toks, e][:, None] * yg.T
    for e in range(E):
        out[e * TOKS_SH:(e + 1) * TOKS_SH] += res.results[e]["ys"].T
    return out.reshape(B, S, H), viol
